# revision 4
# baseline (speedup 1.0000x reference)
"""MiniBatchSemiNMF encode kernel for Trainium2 (8 NeuronCores, Bass/Tile).

Data-parallel over the batch (rows of `acts`): each of the 8 cores gets
1024 rows; D-derived k x k cache terms (ddt_pos, ddt_neg, (ddt+eps I)^-1)
are computed on the host (tiny: 512x512) and replicated to every core.

Device computation per core, in a transposed layout (k on partitions,
rows on the free dim), so no on-device transposes are needed:
    atdT  = D @ actsT                                  (PE, fp32r)
    z0T   = max(inv @ atdT, eps)                       (PE + DVE)
    loop: numT = atd_posT     + ddt_neg @ zT           (PE, identity-matmul
          denT = atd_negT+eps + ddt_pos @ zT            folds the adds into
          zT  *= sqrt(numT) * rsqrt(denT)               the PSUM group)
Iteration matmuls run as fp32r (full PE rate; ~12-bit mantissa products,
fp32 accumulate) -- iteration noise is contracted by the dynamics. The
atd/z0 matmuls run in exact fp32 (4x cycles, but only 96 of 1696 MMs):
their rounding would persist in z as an initial-condition error. Elementwise runs on ACT (sqrt/rsqrt/relu) and DVE (mul/add).
"""

import sys

for _p in ("/opt/trn_rl_repo",):
    if _p not in sys.path:
        sys.path.insert(0, _p)

import numpy as np

import concourse.bacc as bacc
import concourse.tile as tile
from concourse import mybir
from concourse.bass_utils import run_bass_kernel_spmd

F32 = mybir.dt.float32
F32R = mybir.dt.float32r

EPS = 1e-8
N_CORES = 8
B, DM, K = 8192, 1024, 512  # batch, d_model, n_concepts
R = B // N_CORES  # rows per core (1024)
RC = 512  # row-chunk (moving-operand width)
NRC = R // RC  # 2 row chunks
NK = K // 128  # 4 k-tiles
ND = DM // 128  # 8 d-tiles

_BUILD_CACHE: dict[int, object] = {}


def _build(n_iters: int):
    """Build (and bacc-compile) the per-core Bass program."""
    nc = bacc.Bacc("TRN2", target_bir_lowering=False, debug=False, num_devices=N_CORES)

    actsT_d = nc.dram_tensor("actsT", [DM, R], F32, kind="ExternalInput").ap()
    DT_d = nc.dram_tensor("DT", [DM, K], F32, kind="ExternalInput").ap()
    dpos_d = nc.dram_tensor("ddt_pos", [K, K], F32R, kind="ExternalInput").ap()
    dneg_d = nc.dram_tensor("ddt_neg", [K, K], F32R, kind="ExternalInput").ap()
    inv_d = nc.dram_tensor("ddt_inv", [K, K], F32, kind="ExternalInput").ap()
    eye_d = nc.dram_tensor("eye", [128, 128], F32R, kind="ExternalInput").ap()
    out_d = nc.dram_tensor("zT", [K, R], F32, kind="ExternalOutput").ap()

    Relu = mybir.ActivationFunctionType.Relu
    Sqrt = mybir.ActivationFunctionType.Sqrt
    Rsqrt = mybir.ActivationFunctionType.Rsqrt
    Copy = mybir.ActivationFunctionType.Copy

    with tile.TileContext(nc) as tc:
        with (
            tc.tile_pool(name="weights", bufs=1) as wp,
            tc.tile_pool(name="big", bufs=1) as bigp,
            tc.tile_pool(name="zacts", bufs=2 * NK * NRC) as zap,
            tc.tile_pool(name="tmp", bufs=4) as tmpp,
            tc.tile_pool(name="psum", bufs=4, space="PSUM") as psp,
        ):
            # --- persistent weights ---
            eye_sb = wp.tile([128, 128], F32R, name="eye_sb", tag="eye")
            nc.sync.dma_start(eye_sb[:], eye_d[:])
            DT_sb = []
            for d in range(ND):
                t = wp.tile([128, K], F32, name=f"DT_sb{d}", tag=f"DT{d}")
                nc.sync.dma_start(t[:], DT_d[d * 128 : (d + 1) * 128, :])
                DT_sb.append(t)
            # actsT tiles share slots with z tiles (tag "za"): actsT is dead
            # after phase 1, exactly when z starts being written.
            acts_sb = [[None] * NRC for _ in range(ND)]
            for d in range(ND):
                for rc in range(NRC):
                    t = zap.tile([128, RC], F32, name=f"acts{d}_{rc}", tag="za")
                    nc.sync.dma_start(
                        t[:], actsT_d[d * 128 : (d + 1) * 128, rc * RC : (rc + 1) * RC]
                    )
                    acts_sb[d][rc] = t
            inv_sb, dpos_sb, dneg_sb = [], [], []
            for k in range(NK):
                rows = slice(k * 128, (k + 1) * 128)
                t = wp.tile([128, K], F32, name=f"inv_sb{k}", tag=f"inv{k}")
                nc.sync.dma_start(t[:], inv_d[rows, :])
                inv_sb.append(t)
                t = wp.tile([128, K], F32R, name=f"dpos_sb{k}", tag=f"dpos{k}")
                nc.sync.dma_start(t[:], dpos_d[rows, :])
                dpos_sb.append(t)
                t = wp.tile([128, K], F32R, name=f"dneg_sb{k}", tag=f"dneg{k}")
                nc.sync.dma_start(t[:], dneg_d[rows, :])
                dneg_sb.append(t)

            # --- phase 1: atdT = D @ actsT, then relu splits ---
            atd_sb = [[None] * NRC for _ in range(NK)]
            pos_sb = [[None] * NRC for _ in range(NK)]
            negeps_sb = [[None] * NRC for _ in range(NK)]
            for kp in range(NK):
                cols = slice(kp * 128, (kp + 1) * 128)
                for rc in range(NRC):
                    ps = psp.tile([128, RC], F32, name=f"ps_atd{kp}_{rc}", tag="pn")
                    for d in range(ND):
                        nc.tensor.matmul(
                            ps[:],
                            DT_sb[d][:, cols],
                            acts_sb[d][rc][:],
                            start=(d == 0),
                            stop=(d == ND - 1),
                        )
                    atd = bigp.tile([128, RC], F32, name=f"atd{kp}_{rc}", tag=f"atd{kp}_{rc}")
                    nc.scalar.activation(atd[:], ps[:], Copy)
                    pos = bigp.tile([128, RC], F32R, name=f"pos{kp}_{rc}", tag=f"pos{kp}_{rc}")
                    nc.scalar.activation(pos[:], ps[:], Relu)
                    neg = tmpp.tile([128, RC], F32, name=f"neg{kp}_{rc}", tag="negt")
                    nc.scalar.activation(neg[:], ps[:], Relu, scale=-1.0)
                    nege = bigp.tile(
                        [128, RC], F32R, name=f"nege{kp}_{rc}", tag=f"nege{kp}_{rc}"
                    )
                    nc.vector.tensor_scalar_add(nege[:], neg[:], EPS)
                    atd_sb[kp][rc] = atd
                    pos_sb[kp][rc] = pos
                    negeps_sb[kp][rc] = nege

            # --- phase 2: z0T = max(inv @ atdT, eps) ---
            z_sb = [[[None] * NRC for _ in range(NK)] for _ in range(2)]
            for p in range(2):
                for k in range(NK):
                    for rc in range(NRC):
                        z_sb[p][k][rc] = zap.tile(
                            [128, RC], F32R, name=f"z{p}_{k}_{rc}", tag="za"
                        )
            for kp in range(NK):
                cols = slice(kp * 128, (kp + 1) * 128)
                for rc in range(NRC):
                    ps = psp.tile([128, RC], F32, name=f"ps_z0{kp}_{rc}", tag="pd")
                    for k in range(NK):
                        nc.tensor.matmul(
                            ps[:],
                            inv_sb[k][:, cols],
                            atd_sb[k][rc][:],
                            start=(k == 0),
                            stop=(k == NK - 1),
                        )
                    nc.vector.tensor_scalar_max(z_sb[0][kp][rc][:], ps[:], EPS)

            # --- phase 3: multiplicative updates ---
            for t_it in range(n_iters):
                cur, nxt = t_it % 2, (t_it + 1) % 2
                for rc in range(NRC):
                    for kp in range(NK):
                        cols = slice(kp * 128, (kp + 1) * 128)
                        pn = psp.tile(
                            [128, RC], F32, name=f"pn{t_it}_{rc}_{kp}", tag="pn"
                        )
                        nc.tensor.matmul(
                            pn[:], eye_sb[:], pos_sb[kp][rc][:], start=True, stop=False
                        )
                        for k in range(NK):
                            nc.tensor.matmul(
                                pn[:],
                                dneg_sb[k][:, cols],
                                z_sb[cur][k][rc][:],
                                start=False,
                                stop=(k == NK - 1),
                            )
                        pd = psp.tile(
                            [128, RC], F32, name=f"pd{t_it}_{rc}_{kp}", tag="pd"
                        )
                        nc.tensor.matmul(
                            pd[:], eye_sb[:], negeps_sb[kp][rc][:], start=True, stop=False
                        )
                        for k in range(NK):
                            nc.tensor.matmul(
                                pd[:],
                                dpos_sb[k][:, cols],
                                z_sb[cur][k][rc][:],
                                start=False,
                                stop=(k == NK - 1),
                            )
                        rcp = tmpp.tile(
                            [128, RC], F32, name=f"rcp{t_it}_{rc}_{kp}", tag="rcp"
                        )
                        nc.vector.reciprocal_approx_fast(rcp[:], pd[:])
                        rat = tmpp.tile(
                            [128, RC], F32, name=f"rat{t_it}_{rc}_{kp}", tag="rat"
                        )
                        nc.vector.tensor_mul(rat[:], pn[:], rcp[:])
                        f = tmpp.tile([128, RC], F32, name=f"f{t_it}_{rc}_{kp}", tag="f")
                        nc.scalar.activation(f[:], rat[:], Sqrt)
                        nc.gpsimd.tensor_mul(
                            z_sb[nxt][kp][rc][:],
                            z_sb[cur][kp][rc][:].bitcast(F32),
                            f[:],
                        )

            # --- output ---
            fin = n_iters % 2
            for kp in range(NK):
                for rc in range(NRC):
                    nc.sync.dma_start(
                        out_d[kp * 128 : (kp + 1) * 128, rc * RC : (rc + 1) * RC],
                        z_sb[fin][kp][rc][:].bitcast(F32),
                    )

    nc.compile()
    return nc


def _get_program(n_iters: int):
    if n_iters not in _BUILD_CACHE:
        _BUILD_CACHE[n_iters] = _build(n_iters)
    return _BUILD_CACHE[n_iters]


def make_in_maps(acts: np.ndarray, D: np.ndarray):
    """Host-side sharding + kxk cache terms."""
    acts = np.ascontiguousarray(acts, dtype=np.float32)
    D = np.ascontiguousarray(D, dtype=np.float32)
    ddt = D @ D.T
    ddt_pos = ((np.abs(ddt) + ddt) * 0.5).astype(np.float32)
    ddt_neg = ((np.abs(ddt) - ddt) * 0.5).astype(np.float32)
    eye_k = np.eye(K, dtype=np.float32)
    inv = np.linalg.solve(ddt + np.float32(EPS) * eye_k, eye_k).astype(np.float32)
    DT = np.ascontiguousarray(D.T)
    actsT = np.ascontiguousarray(acts.T)
    eye128 = np.eye(128, dtype=np.float32)
    in_maps = []
    for c in range(N_CORES):
        in_maps.append(
            {
                "actsT": np.ascontiguousarray(actsT[:, c * R : (c + 1) * R]),
                "DT": DT,
                "ddt_pos": ddt_pos,
                "ddt_neg": ddt_neg,
                "ddt_inv": inv,
                "eye": eye128,
            }
        )
    return in_maps


def kernel(acts: np.ndarray, D: np.ndarray, n_iters) -> np.ndarray:
    n_iters = int(n_iters)
    nc = _get_program(n_iters)
    in_maps = make_in_maps(acts, D)
    res = run_bass_kernel_spmd(nc, in_maps, core_ids=list(range(N_CORES)))
    z = np.empty((B, K), dtype=np.float32)
    for c in range(N_CORES):
        z[c * R : (c + 1) * R, :] = res.results[c]["zT"].T
    return z
